# revision 38
# baseline (speedup 1.0000x reference)
"""GQA attention kernel (B=1, S=2048, D=4096, 32 Q heads / 8 KV heads, RoPE,
causal) for 8 Trainium2 NeuronCores.

Sharding: tensor-parallel over heads. Core c owns Q heads 4c..4c+3 and KV head
c (whole GQA group), computes its context slice and a partial o-projection
(rows 512c..512c+511 of Wo); the host sums the 8 partial outputs.

All PE inputs are bf16 (rel err ~5e-3 end-to-end, validated vs the fp32
reference on CPU); PSUM accumulation is fp32. Layout keeps activations
feature-on-partition: hsT [D, S], qT/kT/vT [128, S].

Pipeline (per 512-query chunk c): QKV(c) -> o-proj(c-1) -> RoPE(c) ->
attention(c). The o-projection consumes the previous chunk's context straight
from SBUF, so its matmuls fill the PE while the DVE runs RoPE for chunk c,
and the output DMA is spread across the whole kernel instead of a tail burst.

Softmax: no max-subtraction (logits are O(10), exp safe in fp32); exp on the
Act engine -> bf16 P tiles; causal masking is a 0/1 multiply on diagonal
tiles; row-sums accumulate P tiles into two f32r partials on the DVE (two
chains so the adds keep pace with the PE) and one ones-matmul pair reduces
them across partitions with the result broadcast to all 128 partitions, so
the reciprocal runs full-width (the old [1,512] reciprocal cost 3.3us each).
"""
import numpy as np
import ml_dtypes
from contextlib import ExitStack

try:  # reuse compiled executables across processes when possible
    import jax
    jax.config.update("jax_compilation_cache_dir", "/tmp/jax_comp_cache")
    jax.config.update("jax_persistent_cache_min_entry_size_bytes", -1)
    jax.config.update("jax_persistent_cache_min_compile_time_secs", 1.0)
except Exception:
    pass

import concourse.bacc as bacc
import concourse.tile as tile
import concourse.mybir as mybir
from concourse.bass_utils import run_bass_kernel_spmd

F32 = mybir.dt.float32
F32R = mybir.dt.float32r
BF16 = mybir.dt.bfloat16
BF = ml_dtypes.bfloat16

S = 2048            # sequence length
D = 4096            # hidden dim
HD = 128            # head dim
NCORES = 8
QH = 4              # q heads per core
KT = D // 128       # 32 contraction tiles for the projections
NCHUNK = S // 512   # 4 sequence chunks of 512
INV_SQRT_D = float(1.0 / np.sqrt(np.float32(HD)))
ROPE_BASE = 10000.0


def _build_nc():
    nc = bacc.Bacc(None)

    hst_d = nc.dram_tensor("hst", [D, S], BF16, kind="ExternalInput")
    wq_d = nc.dram_tensor("wq", [D, QH * HD], BF16, kind="ExternalInput")
    wk_d = nc.dram_tensor("wk", [D, HD], BF16, kind="ExternalInput")
    wv_d = nc.dram_tensor("wv", [D, HD], BF16, kind="ExternalInput")
    wo_d = nc.dram_tensor("wo", [QH * HD, D], BF16, kind="ExternalInput")
    cos_d = nc.dram_tensor("cost", [HD, S], BF16, kind="ExternalInput")
    sin_d = nc.dram_tensor("sint", [HD, S], BF16, kind="ExternalInput")
    mask_d = nc.dram_tensor("maskt", [128, 4, 512], BF16, kind="ExternalInput")
    rt_d = nc.dram_tensor("rt", [128, 128], BF16, kind="ExternalInput")
    ident_d = nc.dram_tensor("ident", [128, 128], F32R, kind="ExternalInput")
    ones_d = nc.dram_tensor("ones", [128, 128], F32R, kind="ExternalInput")
    out_d = nc.dram_tensor("out", [S, D], BF16, kind="ExternalOutput")

    with tile.TileContext(nc) as tc, ExitStack() as ctx:
        wpool = ctx.enter_context(tc.tile_pool(name="wpool", bufs=1))
        cpool = ctx.enter_context(tc.tile_pool(name="cpool", bufs=1))
        big = ctx.enter_context(tc.tile_pool(name="bigacts", bufs=1))
        hsp = ctx.enter_context(tc.tile_pool(name="hsp", bufs=34))
        evv = ctx.enter_context(tc.tile_pool(name="evv", bufs=2))
        evac = ctx.enter_context(tc.tile_pool(name="evac", bufs=6))
        qrp = ctx.enter_context(tc.tile_pool(name="qrp", bufs=8))
        tmp = ctx.enter_context(tc.tile_pool(name="tmp", bufs=4))
        ptp = ctx.enter_context(tc.tile_pool(name="ptp", bufs=7))
        pad = ctx.enter_context(tc.tile_pool(name="pad", bufs=4))
        rcp = ctx.enter_context(tc.tile_pool(name="rcp", bufs=2))
        osb = ctx.enter_context(tc.tile_pool(name="osb", bufs=3))
        psum = ctx.enter_context(tc.tile_pool(name="psum", bufs=8, space="PSUM"))

        # ---- resident weights & constants ----
        wq_sb = wpool.tile([128, KT, QH * HD], BF16, tag="wq")
        wq_r = wq_d[:, :].rearrange("(t p) m -> p t m", p=128)
        wk_sb = wpool.tile([128, KT, HD], BF16, tag="wk")
        wk_r = wk_d[:, :].rearrange("(t p) m -> p t m", p=128)
        wv_sb = wpool.tile([128, KT, HD], BF16, tag="wv")
        wv_r = wv_d[:, :].rearrange("(t p) m -> p t m", p=128)
        wo_sb = wpool.tile([128, QH, D], BF16, tag="wo")
        wo_r = wo_d[:, :].rearrange("(t p) e -> p t e", p=128)

        # minimal first slices (on separate queues) so the PE starts fast;
        # the very first hst tile is split across two queues since it gates
        # the first matmul of the whole kernel
        hst_t00 = hsp.tile([128, 512], BF16, tag="hst")
        nc.sync.dma_start(out=hst_t00[0:64, :], in_=hst_d[0:64, 0:512])
        nc.scalar.dma_start(out=hst_t00[64:128, :], in_=hst_d[64:128, 0:512])
        nc.gpsimd.dma_start(out=wk_sb[:, 0:1, :], in_=wk_r[:, 0:1, :])
        nc.gpsimd.dma_start(out=wv_sb[:, 0:1, :], in_=wv_r[:, 0:1, :])
        nc.scalar.dma_start(out=wq_sb[:, 0:1, :], in_=wq_r[:, 0:1, :])

        cos_sb = cpool.tile([HD, S], BF16, tag="cos")
        sin_sb = cpool.tile([HD, S], BF16, tag="sin")
        mask_sb = cpool.tile([128, 4, 512], BF16, tag="mask")
        rt_sb = cpool.tile([128, 128], BF16, tag="rt")
        ident_sb = cpool.tile([128, 128], F32R, tag="ident")
        ones_sb = cpool.tile([128, 128], F32R, tag="ones")

        # remaining resident loads, all issued upfront: the queues transfer in
        # order and fan out across the 16 DMA engines, so arrival outpaces the
        # PE's consumption slope (first bench starved the PE for 15us when
        # these were staggered into the t-loop)
        nc.scalar.dma_start(out=wq_sb[:, 1:8, :], in_=wq_r[:, 1:8, :])
        nc.scalar.dma_start(out=wq_sb[:, 8:16, :], in_=wq_r[:, 8:16, :])
        nc.scalar.dma_start(out=wq_sb[:, 16:24, :], in_=wq_r[:, 16:24, :])
        nc.scalar.dma_start(out=wq_sb[:, 24:32, :], in_=wq_r[:, 24:32, :])
        nc.gpsimd.dma_start(out=wk_sb[:, 1:16, :], in_=wk_r[:, 1:16, :])
        nc.gpsimd.dma_start(out=wv_sb[:, 1:16, :], in_=wv_r[:, 1:16, :])
        nc.gpsimd.dma_start(out=wk_sb[:, 16:32, :], in_=wk_r[:, 16:32, :])
        nc.gpsimd.dma_start(out=wv_sb[:, 16:32, :], in_=wv_r[:, 16:32, :])
        nc.gpsimd.dma_start(out=cos_sb[:], in_=cos_d[:, :])
        nc.gpsimd.dma_start(out=sin_sb[:], in_=sin_d[:, :])
        nc.gpsimd.dma_start(out=rt_sb[:], in_=rt_d[:, :])
        nc.gpsimd.dma_start(out=ident_sb[:], in_=ident_d[:, :])
        nc.gpsimd.dma_start(out=ones_sb[:], in_=ones_d[:, :])
        nc.gpsimd.dma_start(out=mask_sb[:], in_=mask_d[:, :, :])
        # wo is deferred past chunk 0's QKV: the startup window already runs
        # at the HBM limit across 8 cores and wo isn't needed until ~150us

        krope_sb = big.tile([128, S], BF16, tag="krope")   # kT after rope
        vnat_sb = big.tile([128, S], BF16, tag="vnat")     # v natural [j, d] blocks
        ctx_sb = big.tile([128, QH, S], BF16, tag="ctx")   # normalized context^T

        def _emit_oproj_quarter(icnk, st):
            # one quarter (128 output rows) of the o-projection for chunk
            # icnk: out[s, e] = sum_h ctx_h[d, s]^T wo_h[d, e]. Quarters are
            # interleaved between attention heads of the NEXT chunk so these
            # act-engine-free matmuls fill the PE while the exp stream drains
            # (the act engine is the throughput wall inside attention).
            # Evacuations alternate DVE/GpSimd, never the act engine.
            c0 = icnk * 512
            row0 = c0 + st * 128
            for eo in range(4):
                ot = osb.tile([128, 1024], BF16, tag="ot",
                              name=f"ot{icnk}_{st}_{eo}")
                for half in range(2):
                    ec = eo * 2 + half
                    oacc = psum.tile([128, 512], F32, tag="ps",
                                     name=f"oacc{icnk}_{st}_{ec}")
                    for h in range(QH):
                        nc.tensor.matmul(oacc[:],
                                         ctx_sb[:, h, row0:row0 + 128],
                                         wo_sb[:, h, ec * 512:(ec + 1) * 512],
                                         start=(h == 0), stop=(h == 3))
                    # pool/gpsimd cannot read PSUM; split copies DVE/Act so
                    # neither engine's attention-phase stream is overwhelmed
                    if ec % 2 == 0:
                        nc.vector.tensor_copy(ot[:, half * 512:(half + 1) * 512], oacc[:])
                    else:
                        nc.scalar.copy(ot[:, half * 512:(half + 1) * 512], oacc[:])
                dma_eng = nc.gpsimd if eo % 2 == 0 else nc.sync
                dma_eng.dma_start(
                    out=out_d[row0:row0 + 128, eo * 1024:(eo + 1) * 1024],
                    in_=ot[:])

        # ---- fused pipeline, explicit phase order ----
        # QKV(0) -> QKV(1) -> attn(0) -> attn(1)+oproj(0)/4 -> QKV(2) ->
        # attn(2)+oproj(1)/4 -> QKV(3) -> attn(3)+oproj(2)/4 -> oproj(3).
        # QKV(1) is emitted before attn(0) so chunk 0's rope chain and
        # attention exp stream hide behind 40us of projection matmuls (the
        # sync queue is in-order, so chunk 1's hst prefetch cannot delay
        # chunk 0's tiles).
        _PENDING = []  # deferred per-head normalization (ctx, padd, h, c0, c1)
        _QROPE = {}

        def _finish_head(ctx_acc, padd, h, cc0, cc1):
            # rowsum broadcast to all partitions via ones-matmul, then a
            # full-width reciprocal; deferred past the next head's first
            # scores so the rowsum matmul never stalls on the padd chains
            rs = psum.tile([128, 512], F32, tag="ps")
            nc.tensor.matmul(rs[:], ones_sb[:], padd[0][:], start=True, stop=False)
            nc.tensor.matmul(rs[:], ones_sb[:], padd[1][:], start=False, stop=True)
            recip = rcp.tile([128, 512], F32, tag="rcp")
            nc.vector.reciprocal_approx_fast(out=recip[:], in_=rs[:])
            nc.vector.tensor_mul(ctx_sb[:, h, cc0:cc1], ctx_acc[:], recip[:])

        def _emit_qkv_rope(icnk):
            c0, c1 = icnk * 512, (icnk + 1) * 512

            # rope phase B: elementwise q*cos + rot(q)*sin on the DVE
            def _rope_b(ch, rot, m):
                t1 = tmp.tile([128, 512], BF16, tag="t1", name=f"t1_{icnk}_{m}")
                nc.vector.tensor_mul(t1[:], ch[:], cos_sb[:, c0:c1])
                t2 = tmp.tile([128, 512], BF16, tag="t2", name=f"t2_{icnk}_{m}")
                nc.vector.tensor_mul(t2[:], rot[:], sin_sb[:, c0:c1])
                if m < 0:
                    nc.vector.tensor_add(krope_sb[:, c0:c1], t1[:], t2[:])
                    return None
                dest = qrp.tile([128, 512], BF16, tag="qrp", name=f"qr{icnk}_{m}")
                nc.vector.tensor_add(dest[:], t1[:], t2[:])
                return dest

            def _rope_k(acc_k):
                ch_k = evac.tile([128, 512], BF16, tag="evac", name=f"chk{icnk}")
                nc.vector.tensor_copy(ch_k[:], acc_k[:])
                rot_k = psum.tile([128, 512], F32, tag="ps", name=f"rotk{icnk}")
                nc.tensor.matmul(rot_k[:], rt_sb[:], ch_k[:], start=True, stop=True)
                _rope_b(ch_k, rot_k, -1)

            def _transp_v(acc_v):
                chv = evv.tile([128, 512], F32R, tag="evacv", name=f"chv{icnk}")
                nc.vector.tensor_copy(chv[:], acc_v[:])
                for tt in range(4):
                    jt = icnk * 4 + tt
                    vt_ps = psum.tile([128, 128], F32R, tag="ps",
                                      name=f"vt{icnk}_{tt}")
                    nc.tensor.matmul(vt_ps[:], chv[:, tt * 128:(tt + 1) * 128],
                                     ident_sb[:], is_transpose=True,
                                     start=True, stop=True)
                    nc.vector.tensor_copy(vnat_sb[:, jt * 128:(jt + 1) * 128],
                                          vt_ps[:])

            def _rope_q(acc, m):
                ch = evac.tile([128, 512], BF16, tag="evac", name=f"chq{icnk}_{m}")
                nc.vector.tensor_copy(ch[:], acc[:])
                rot = psum.tile([128, 512], F32, tag="ps", name=f"rotq{icnk}_{m}")
                nc.tensor.matmul(rot[:], rt_sb[:], ch[:], start=True, stop=True)
                return _rope_b(ch, rot, m)

            if icnk <= 1:
                # chunks 0/1 are DMA-paced (their hst loads overlap the
                # startup weight crunch): interleave the six projections per
                # t-tile so each hst tile is consumed as soon as it arrives
                accs = [psum.tile([128, 512], F32, tag="ps",
                                  name=f"acc{icnk}_{i}") for i in range(6)]
                for t in range(KT):
                    if icnk == 0 and t == 0:
                        hst_t = hst_t00  # preloaded before the weights
                    else:
                        hst_t = hsp.tile([128, 512], BF16, tag="hst")
                        nc.sync.dma_start(out=hst_t[:],
                                          in_=hst_d[t * 128:(t + 1) * 128, c0:c1])
                    for i, m in enumerate((4, 5, 0, 1, 2, 3)):
                        if m < 4:
                            lhsT = wq_sb[:, t, m * HD:(m + 1) * HD]
                        elif m == 4:
                            lhsT = wk_sb[:, t, :]
                        else:
                            lhsT = wv_sb[:, t, :]
                        nc.tensor.matmul(accs[i][:], lhsT, hst_t[:],
                                         start=(t == 0), stop=(t == KT - 1))
                _rope_k(accs[0])
                qrope_chunks = [_rope_q(accs[2], 0)]
                _transp_v(accs[1])
                qrope_chunks += [_rope_q(accs[2 + m], m) for m in range(1, 4)]
                if icnk == 0:
                    # wo load deferred out of the startup HBM crunch
                    nc.gpsimd.dma_start(out=wo_sb[:, 0:2, :], in_=wo_r[:, 0:2, :])
                    nc.gpsimd.dma_start(out=wo_sb[:, 2:4, :], in_=wo_r[:, 2:4, :])
            else:
                # later chunks: hst is prefetched ahead of time, so run one
                # projection PASS at a time over the resident tiles: each
                # accumulator stops 32 matmuls in and its rope/evac overlaps
                # the remaining passes
                hst_tiles = []
                for t in range(KT):
                    hst_t = hsp.tile([128, 512], BF16, tag="hst")
                    nc.sync.dma_start(out=hst_t[:],
                                      in_=hst_d[t * 128:(t + 1) * 128, c0:c1])
                    hst_tiles.append(hst_t)
                qrope_chunks = []
                for m in (4, 5, 0, 1, 2, 3):
                    acc = psum.tile([128, 512], F32, tag="ps",
                                    name=f"acc{icnk}_{m}")
                    for t in range(KT):
                        if m < 4:
                            lhsT = wq_sb[:, t, m * HD:(m + 1) * HD]
                        elif m == 4:
                            lhsT = wk_sb[:, t, :]
                        else:
                            lhsT = wv_sb[:, t, :]
                        nc.tensor.matmul(acc[:], lhsT, hst_tiles[t][:],
                                         start=(t == 0), stop=(t == KT - 1))
                    if m == 4:
                        _rope_k(acc)
                    elif m == 5:
                        _transp_v(acc)
                    else:
                        qrope_chunks.append(_rope_q(acc, m))
            _QROPE[icnk] = qrope_chunks

        def _emit_attn(icnk, oproj_of):
            # attention for the 4 heads, query chunk = icnk (keys 0..4*icnk+3)
            c0, c1 = icnk * 512, (icnk + 1) * 512
            qrope_chunks = _QROPE.pop(icnk)
            jt_max = icnk * 4 + 3
            for h in range(QH):
                qr = qrope_chunks[h]
                ctx_acc = psum.tile([128, 512], F32, tag="ps")
                padd = [pad.tile([128, 512], F32R, tag="pad",
                                 name=f"padd{icnk}_{h}_{i}") for i in range(2)]
                # software-pipelined: the av matmul for tile jt is emitted
                # after the scores matmul of jt+4, so the (in-order) PE never
                # waits on the exp -> mask -> padd chain
                pending = []
                LOOKAHEAD = 4

                def _consume(pjt, ppT, pn0, last):
                    # diagonal av matmuls skip the masked-out (zero) query
                    # columns; the untouched psum region keeps its value from
                    # the other jt tiles (skip_group_check: partial-region
                    # start/stop bookkeeping is sim-only)
                    nc.tensor.matmul(ctx_acc[:, pn0:512],
                                     vnat_sb[:, pjt * 128:(pjt + 1) * 128],
                                     ppT[:, pn0:512], start=(pjt == 0), stop=last,
                                     skip_group_check=(pn0 > 0 or last))

                def _emit_padd(pjt, ppT):
                    # row-sum partials: two alternating f32r chains on the
                    # DVE, deferred one jt step so the diagonal mask multiply
                    # (which gates the av matmul) never queues behind them
                    p = padd[pjt % 2]
                    if pjt < 2:
                        nc.vector.tensor_copy(p[:], ppT[:])
                    else:
                        nc.vector.tensor_add(p[:], p[:], ppT[:])

                for jt in range(jt_max + 1):
                    # diagonal tiles only produce scores for queries >= 128*r;
                    # the matmul/exp are trimmed to that range (the mask
                    # multiply still covers the whole tile, zeroing the stale
                    # region). chunk 0 stays full-width so every pT pool
                    # buffer is written before any trimmed use.
                    r = jt - icnk * 4
                    n0 = 128 * r if (icnk > 0 and r >= 1) else 0
                    sT = psum.tile([128, 512], F32, tag="ps")
                    nc.tensor.matmul(sT[:, n0:512], krope_sb[:, jt * 128:(jt + 1) * 128],
                                     qr[:, n0:512], start=True, stop=True)
                    if jt == 1 and _PENDING:
                        _finish_head(*_PENDING.pop())
                    if len(pending) >= LOOKAHEAD:
                        _consume(*pending.pop(0), False)
                    pT = ptp.tile([128, 512], BF16, tag="pt")
                    nc.scalar.activation(out=pT[:, n0:512], in_=sT[:, n0:512],
                                         func=mybir.ActivationFunctionType.Exp,
                                         scale=INV_SQRT_D)
                    if r >= 0:
                        nc.vector.tensor_mul(pT[:], pT[:], mask_sb[:, r, :])
                    if jt > 0:
                        _emit_padd(jt - 1, pending[-1][1])
                    pending.append((jt, pT, n0))
                _emit_padd(jt_max, pT)
                while pending:
                    _consume(*pending.pop(0), len(pending) == 0)
                _PENDING.append((ctx_acc, padd, h, c0, c1))
                if oproj_of is not None:
                    _emit_oproj_quarter(oproj_of, h)

        _emit_qkv_rope(0)
        _emit_qkv_rope(1)
        _emit_attn(0, None)
        _emit_attn(1, 0)
        _emit_qkv_rope(2)
        _emit_attn(2, 1)
        _emit_qkv_rope(3)
        _emit_attn(3, 2)
        _finish_head(*_PENDING.pop())
        for st in range(4):
            _emit_oproj_quarter(NCHUNK - 1, st)

    nc.finalize()
    return nc


_NC_CACHE = None
_TABLES_CACHE = None


def _host_tables():
    inv_freq = 1.0 / (ROPE_BASE ** (np.arange(0, HD, 2, dtype=np.float32) / HD))
    pos = np.arange(S, dtype=np.float32)
    freqs = pos[:, None] * inv_freq[None, :].astype(np.float32)   # [S, 64]
    emb = np.concatenate([freqs, freqs], axis=1).astype(np.float32)  # [S, 128]
    cosT = np.ascontiguousarray(np.cos(emb).T).astype(BF)  # [128, S]
    sinT = np.ascontiguousarray(np.sin(emb).T).astype(BF)

    # multiplicative causal mask for diagonal blocks, transposed [jp, r, if]
    jp = np.arange(128)[:, None, None]
    r = np.arange(4)[None, :, None]
    iF = np.arange(512)[None, None, :]
    mask01 = (r * 128 + jp <= iF).astype(np.float32).astype(BF)

    rt = np.zeros((128, 128), dtype=np.float32)
    idx = np.arange(64)
    rt[idx + 64, idx] = -1.0
    rt[idx, idx + 64] = 1.0
    rt = rt.astype(BF)

    ident = np.eye(128, dtype=np.float32)
    ones = np.ones((128, 128), dtype=np.float32)
    return cosT, sinT, mask01, rt, ident, ones


def kernel(hidden_states, Wq, Wk, Wv, Wo):
    global _NC_CACHE, _TABLES_CACHE
    if _NC_CACHE is None:
        _NC_CACHE = _build_nc()
    nc = _NC_CACHE
    if _TABLES_CACHE is None:
        _TABLES_CACHE = _host_tables()
    cosT, sinT, mask01, rt, ident, ones = _TABLES_CACHE

    hs = np.asarray(hidden_states, dtype=np.float32)
    B = hs.shape[0]
    assert hs.shape == (B, S, D)
    hst = np.ascontiguousarray(hs[0].T).astype(BF)  # [D, S]

    Wq = np.asarray(Wq, dtype=np.float32)
    Wk = np.asarray(Wk, dtype=np.float32)
    Wv = np.asarray(Wv, dtype=np.float32)
    Wo = np.asarray(Wo, dtype=np.float32)

    in_maps = []
    for c in range(NCORES):
        in_maps.append({
            "hst": hst,
            "wq": np.ascontiguousarray(Wq[:, c * QH * HD:(c + 1) * QH * HD]).astype(BF),
            "wk": np.ascontiguousarray(Wk[:, c * HD:(c + 1) * HD]).astype(BF),
            "wv": np.ascontiguousarray(Wv[:, c * HD:(c + 1) * HD]).astype(BF),
            "wo": np.ascontiguousarray(Wo[c * QH * HD:(c + 1) * QH * HD, :]).astype(BF),
            "cost": cosT,
            "sint": sinT,
            "maskt": mask01,
            "rt": rt,
            "ident": ident,
            "ones": ones,
        })

    import os
    trace = os.environ.get("KERNEL_TRACE") == "1"
    if trace:
        try:
            import antenv.axon_hooks  # noqa: F401  (profiling hook, optional)
        except ImportError:
            trace = False
    res = run_bass_kernel_spmd(nc, in_maps, list(range(NCORES)), trace=trace)
    if trace:
        kernel.last_results = res

    acc = np.zeros((S, D), dtype=np.float32)
    for c in range(NCORES):
        acc += res.results[c]["out"].astype(np.float32)
    return acc.reshape(B, S, D)


# revision 39
# speedup vs baseline: 1.1976x; 1.1976x over previous
"""GQA attention kernel (B=1, S=2048, D=4096, 32 Q heads / 8 KV heads, RoPE,
causal) for 8 Trainium2 NeuronCores.

Sharding: tensor-parallel over heads. Core c owns Q heads 4c..4c+3 and KV head
c (whole GQA group), computes its context slice and a partial o-projection
(rows 512c..512c+511 of Wo); the host sums the 8 partial outputs.

All PE inputs are bf16 (rel err ~5e-3 end-to-end, validated vs the fp32
reference on CPU); PSUM accumulation is fp32. Layout keeps activations
feature-on-partition: hsT [D, S], qT/kT/vT [128, S].

Pipeline (per 512-query chunk c): QKV(c) -> o-proj(c-1) -> RoPE(c) ->
attention(c). The o-projection consumes the previous chunk's context straight
from SBUF, so its matmuls fill the PE while the DVE runs RoPE for chunk c,
and the output DMA is spread across the whole kernel instead of a tail burst.

Softmax: no max-subtraction (logits are O(10), exp safe in fp32); exp on the
Act engine -> bf16 P tiles; causal masking is a 0/1 multiply on diagonal
tiles; row-sums accumulate P tiles into two f32r partials on the DVE (two
chains so the adds keep pace with the PE) and one ones-matmul pair reduces
them across partitions with the result broadcast to all 128 partitions, so
the reciprocal runs full-width (the old [1,512] reciprocal cost 3.3us each).
"""
import numpy as np
import ml_dtypes
from contextlib import ExitStack

try:  # reuse compiled executables across processes when possible
    import jax
    jax.config.update("jax_compilation_cache_dir", "/tmp/jax_comp_cache")
    jax.config.update("jax_persistent_cache_min_entry_size_bytes", -1)
    jax.config.update("jax_persistent_cache_min_compile_time_secs", 1.0)
except Exception:
    pass

import concourse.bacc as bacc
import concourse.tile as tile
import concourse.mybir as mybir
from concourse.bass_utils import run_bass_kernel_spmd

F32 = mybir.dt.float32
F32R = mybir.dt.float32r
BF16 = mybir.dt.bfloat16
BF = ml_dtypes.bfloat16

S = 2048            # sequence length
D = 4096            # hidden dim
HD = 128            # head dim
NCORES = 8
QH = 4              # q heads per core
KT = D // 128       # 32 contraction tiles for the projections
NCHUNK = S // 512   # 4 sequence chunks of 512
INV_SQRT_D = float(1.0 / np.sqrt(np.float32(HD)))
ROPE_BASE = 10000.0


def _build_nc():
    nc = bacc.Bacc(None)

    hst_d = nc.dram_tensor("hst", [D, S], BF16, kind="ExternalInput")
    wq_d = nc.dram_tensor("wq", [D, QH * HD], BF16, kind="ExternalInput")
    wk_d = nc.dram_tensor("wk", [D, HD], BF16, kind="ExternalInput")
    wv_d = nc.dram_tensor("wv", [D, HD], BF16, kind="ExternalInput")
    wo_d = nc.dram_tensor("wo", [QH * HD, D], BF16, kind="ExternalInput")
    cos_d = nc.dram_tensor("cost", [HD, S], BF16, kind="ExternalInput")
    sin_d = nc.dram_tensor("sint", [HD, S], BF16, kind="ExternalInput")
    mask_d = nc.dram_tensor("maskt", [128, 4, 512], BF16, kind="ExternalInput")
    rt_d = nc.dram_tensor("rt", [128, 128], BF16, kind="ExternalInput")
    ident_d = nc.dram_tensor("ident", [128, 128], F32R, kind="ExternalInput")
    ones_d = nc.dram_tensor("ones", [128, 128], F32R, kind="ExternalInput")
    out_d = nc.dram_tensor("out", [S, D], BF16, kind="ExternalOutput")

    with tile.TileContext(nc) as tc, ExitStack() as ctx:
        wpool = ctx.enter_context(tc.tile_pool(name="wpool", bufs=1))
        cpool = ctx.enter_context(tc.tile_pool(name="cpool", bufs=1))
        big = ctx.enter_context(tc.tile_pool(name="bigacts", bufs=1))
        hsp = ctx.enter_context(tc.tile_pool(name="hsp", bufs=34))
        evv = ctx.enter_context(tc.tile_pool(name="evv", bufs=2))
        evac = ctx.enter_context(tc.tile_pool(name="evac", bufs=6))
        qrp = ctx.enter_context(tc.tile_pool(name="qrp", bufs=4))
        tmp = ctx.enter_context(tc.tile_pool(name="tmp", bufs=4))
        ptp = ctx.enter_context(tc.tile_pool(name="ptp", bufs=7))
        pad = ctx.enter_context(tc.tile_pool(name="pad", bufs=4))
        rcp = ctx.enter_context(tc.tile_pool(name="rcp", bufs=2))
        osb = ctx.enter_context(tc.tile_pool(name="osb", bufs=3))
        psum = ctx.enter_context(tc.tile_pool(name="psum", bufs=8, space="PSUM"))

        # ---- resident weights & constants ----
        wq_sb = wpool.tile([128, KT, QH * HD], BF16, tag="wq")
        wq_r = wq_d[:, :].rearrange("(t p) m -> p t m", p=128)
        wk_sb = wpool.tile([128, KT, HD], BF16, tag="wk")
        wk_r = wk_d[:, :].rearrange("(t p) m -> p t m", p=128)
        wv_sb = wpool.tile([128, KT, HD], BF16, tag="wv")
        wv_r = wv_d[:, :].rearrange("(t p) m -> p t m", p=128)
        wo_sb = wpool.tile([128, QH, D], BF16, tag="wo")
        wo_r = wo_d[:, :].rearrange("(t p) e -> p t e", p=128)

        # minimal first slices (on separate queues) so the PE starts fast;
        # the very first hst tile is split across two queues since it gates
        # the first matmul of the whole kernel
        hst_t00 = hsp.tile([128, 512], BF16, tag="hst")
        nc.sync.dma_start(out=hst_t00[0:64, :], in_=hst_d[0:64, 0:512])
        nc.scalar.dma_start(out=hst_t00[64:128, :], in_=hst_d[64:128, 0:512])
        nc.gpsimd.dma_start(out=wk_sb[:, 0:1, :], in_=wk_r[:, 0:1, :])
        nc.gpsimd.dma_start(out=wv_sb[:, 0:1, :], in_=wv_r[:, 0:1, :])
        nc.scalar.dma_start(out=wq_sb[:, 0:1, :], in_=wq_r[:, 0:1, :])

        cos_sb = cpool.tile([HD, S], BF16, tag="cos")
        sin_sb = cpool.tile([HD, S], BF16, tag="sin")
        mask_sb = cpool.tile([128, 4, 512], BF16, tag="mask")
        rt_sb = cpool.tile([128, 128], BF16, tag="rt")
        ident_sb = cpool.tile([128, 128], F32R, tag="ident")
        ones_sb = cpool.tile([128, 128], F32R, tag="ones")

        # remaining resident loads, all issued upfront: the queues transfer in
        # order and fan out across the 16 DMA engines, so arrival outpaces the
        # PE's consumption slope (first bench starved the PE for 15us when
        # these were staggered into the t-loop)
        nc.scalar.dma_start(out=wq_sb[:, 1:8, :], in_=wq_r[:, 1:8, :])
        nc.scalar.dma_start(out=wq_sb[:, 8:16, :], in_=wq_r[:, 8:16, :])
        nc.scalar.dma_start(out=wq_sb[:, 16:24, :], in_=wq_r[:, 16:24, :])
        nc.scalar.dma_start(out=wq_sb[:, 24:32, :], in_=wq_r[:, 24:32, :])
        nc.gpsimd.dma_start(out=wk_sb[:, 1:16, :], in_=wk_r[:, 1:16, :])
        nc.gpsimd.dma_start(out=wv_sb[:, 1:16, :], in_=wv_r[:, 1:16, :])
        nc.gpsimd.dma_start(out=wk_sb[:, 16:32, :], in_=wk_r[:, 16:32, :])
        nc.gpsimd.dma_start(out=wv_sb[:, 16:32, :], in_=wv_r[:, 16:32, :])
        nc.gpsimd.dma_start(out=cos_sb[:], in_=cos_d[:, :])
        nc.gpsimd.dma_start(out=sin_sb[:], in_=sin_d[:, :])
        nc.gpsimd.dma_start(out=rt_sb[:], in_=rt_d[:, :])
        nc.gpsimd.dma_start(out=ident_sb[:], in_=ident_d[:, :])
        nc.gpsimd.dma_start(out=ones_sb[:], in_=ones_d[:, :])
        nc.gpsimd.dma_start(out=mask_sb[:], in_=mask_d[:, :, :])
        # wo is deferred past chunk 0's QKV: the startup window already runs
        # at the HBM limit across 8 cores and wo isn't needed until ~150us

        krope_sb = big.tile([128, S], BF16, tag="krope")   # kT after rope
        vnat_sb = big.tile([128, S], BF16, tag="vnat")     # v natural [j, d] blocks
        ctx_sb = big.tile([128, QH, S], BF16, tag="ctx")   # normalized context^T

        def _emit_oproj_quarter(icnk, st):
            # one quarter (128 output rows) of the o-projection for chunk
            # icnk: out[s, e] = sum_h ctx_h[d, s]^T wo_h[d, e]. Quarters are
            # interleaved between attention heads of the NEXT chunk so these
            # act-engine-free matmuls fill the PE while the exp stream drains
            # (the act engine is the throughput wall inside attention).
            # Evacuations alternate DVE/GpSimd, never the act engine.
            c0 = icnk * 512
            row0 = c0 + st * 128
            for eo in range(4):
                ot = osb.tile([128, 1024], BF16, tag="ot",
                              name=f"ot{icnk}_{st}_{eo}")
                for half in range(2):
                    ec = eo * 2 + half
                    oacc = psum.tile([128, 512], F32, tag="ps",
                                     name=f"oacc{icnk}_{st}_{ec}")
                    for h in range(QH):
                        nc.tensor.matmul(oacc[:],
                                         ctx_sb[:, h, row0:row0 + 128],
                                         wo_sb[:, h, ec * 512:(ec + 1) * 512],
                                         start=(h == 0), stop=(h == 3))
                    # pool/gpsimd cannot read PSUM; split copies DVE/Act so
                    # neither engine's attention-phase stream is overwhelmed
                    if ec % 2 == 0:
                        nc.vector.tensor_copy(ot[:, half * 512:(half + 1) * 512], oacc[:])
                    else:
                        nc.scalar.copy(ot[:, half * 512:(half + 1) * 512], oacc[:])
                dma_eng = nc.gpsimd if eo % 2 == 0 else nc.sync
                dma_eng.dma_start(
                    out=out_d[row0:row0 + 128, eo * 1024:(eo + 1) * 1024],
                    in_=ot[:])

        # ---- fused per-chunk pipeline ----
        _PENDING = []  # deferred per-head normalization (ctx, padd, h, c0, c1)
        for icnk in range(NCHUNK):
            c0, c1 = icnk * 512, (icnk + 1) * 512

            # rope phase B: elementwise q*cos + rot(q)*sin on the DVE
            def _rope_b(ch, rot, m):
                t1 = tmp.tile([128, 512], BF16, tag="t1", name=f"t1_{icnk}_{m}")
                nc.vector.tensor_mul(t1[:], ch[:], cos_sb[:, c0:c1])
                t2 = tmp.tile([128, 512], BF16, tag="t2", name=f"t2_{icnk}_{m}")
                nc.vector.tensor_mul(t2[:], rot[:], sin_sb[:, c0:c1])
                if m < 0:
                    nc.vector.tensor_add(krope_sb[:, c0:c1], t1[:], t2[:])
                    return None
                dest = qrp.tile([128, 512], BF16, tag="qrp", name=f"qr{icnk}_{m}")
                nc.vector.tensor_add(dest[:], t1[:], t2[:])
                return dest

            def _rope_k(acc_k):
                ch_k = evac.tile([128, 512], BF16, tag="evac", name=f"chk{icnk}")
                nc.vector.tensor_copy(ch_k[:], acc_k[:])
                rot_k = psum.tile([128, 512], F32, tag="ps", name=f"rotk{icnk}")
                nc.tensor.matmul(rot_k[:], rt_sb[:], ch_k[:], start=True, stop=True)
                _rope_b(ch_k, rot_k, -1)

            def _transp_v(acc_v):
                chv = evv.tile([128, 512], F32R, tag="evacv", name=f"chv{icnk}")
                nc.vector.tensor_copy(chv[:], acc_v[:])
                for tt in range(4):
                    jt = icnk * 4 + tt
                    vt_ps = psum.tile([128, 128], F32R, tag="ps",
                                      name=f"vt{icnk}_{tt}")
                    nc.tensor.matmul(vt_ps[:], chv[:, tt * 128:(tt + 1) * 128],
                                     ident_sb[:], is_transpose=True,
                                     start=True, stop=True)
                    nc.vector.tensor_copy(vnat_sb[:, jt * 128:(jt + 1) * 128],
                                          vt_ps[:])

            def _rope_q(acc, m):
                ch = evac.tile([128, 512], BF16, tag="evac", name=f"chq{icnk}_{m}")
                nc.vector.tensor_copy(ch[:], acc[:])
                rot = psum.tile([128, 512], F32, tag="ps", name=f"rotq{icnk}_{m}")
                nc.tensor.matmul(rot[:], rt_sb[:], ch[:], start=True, stop=True)
                return _rope_b(ch, rot, m)

            if icnk == 0:
                # chunk 0 is DMA-paced: interleave the six projections per
                # t-tile so each hst tile is consumed as soon as it arrives
                accs = [psum.tile([128, 512], F32, tag="ps", name=f"acc0_{i}")
                        for i in range(6)]
                for t in range(KT):
                    if t == 0:
                        hst_t = hst_t00  # preloaded before the weights
                    else:
                        hst_t = hsp.tile([128, 512], BF16, tag="hst")
                        nc.sync.dma_start(out=hst_t[:],
                                          in_=hst_d[t * 128:(t + 1) * 128, c0:c1])
                    for i, m in enumerate((4, 5, 0, 1, 2, 3)):
                        if m < 4:
                            lhsT = wq_sb[:, t, m * HD:(m + 1) * HD]
                        elif m == 4:
                            lhsT = wk_sb[:, t, :]
                        else:
                            lhsT = wv_sb[:, t, :]
                        nc.tensor.matmul(accs[i][:], lhsT, hst_t[:],
                                         start=(t == 0), stop=(t == KT - 1))
                _rope_k(accs[0])
                qrope_chunks = [_rope_q(accs[2], 0)]
                _transp_v(accs[1])
                qrope_chunks += [_rope_q(accs[2 + m], m) for m in range(1, 4)]
                # wo load deferred out of the startup HBM crunch
                nc.gpsimd.dma_start(out=wo_sb[:, 0:2, :], in_=wo_r[:, 0:2, :])
                nc.gpsimd.dma_start(out=wo_sb[:, 2:4, :], in_=wo_r[:, 2:4, :])
            else:
                # chunks 1-3: hst was fully prefetched during the previous
                # attention phase, so run one projection PASS at a time over
                # the resident tiles: each accumulator stops 32 matmuls in,
                # its rope/evac overlaps the remaining passes, and attention
                # can start immediately after the last pass
                hst_tiles = []
                for t in range(KT):
                    hst_t = hsp.tile([128, 512], BF16, tag="hst")
                    nc.sync.dma_start(out=hst_t[:],
                                      in_=hst_d[t * 128:(t + 1) * 128, c0:c1])
                    hst_tiles.append(hst_t)
                qrope_chunks = []
                for m in (4, 5, 0, 1, 2, 3):
                    acc = psum.tile([128, 512], F32, tag="ps",
                                    name=f"acc{icnk}_{m}")
                    for t in range(KT):
                        if m < 4:
                            lhsT = wq_sb[:, t, m * HD:(m + 1) * HD]
                        elif m == 4:
                            lhsT = wk_sb[:, t, :]
                        else:
                            lhsT = wv_sb[:, t, :]
                        nc.tensor.matmul(acc[:], lhsT, hst_tiles[t][:],
                                         start=(t == 0), stop=(t == KT - 1))
                    if m == 4:
                        _rope_k(acc)
                    elif m == 5:
                        _transp_v(acc)
                    else:
                        qrope_chunks.append(_rope_q(acc, m))

            # attention for the 4 heads, query chunk = icnk (keys 0..4*icnk+3)
            jt_max = icnk * 4 + 3

            def _finish_head(ctx_acc, padd, h, cc0, cc1):
                # rowsum broadcast to all partitions via ones-matmul, then a
                # full-width reciprocal; deferred past the next head's first
                # scores so the rowsum matmul never stalls on the padd chains
                rs = psum.tile([128, 512], F32, tag="ps")
                nc.tensor.matmul(rs[:], ones_sb[:], padd[0][:], start=True, stop=False)
                nc.tensor.matmul(rs[:], ones_sb[:], padd[1][:], start=False, stop=True)
                recip = rcp.tile([128, 512], F32, tag="rcp")
                nc.vector.reciprocal_approx_fast(out=recip[:], in_=rs[:])
                nc.vector.tensor_mul(ctx_sb[:, h, cc0:cc1], ctx_acc[:], recip[:])

            for h in range(QH):
                qr = qrope_chunks[h]
                ctx_acc = psum.tile([128, 512], F32, tag="ps")
                padd = [pad.tile([128, 512], F32R, tag="pad",
                                 name=f"padd{icnk}_{h}_{i}") for i in range(2)]
                # software-pipelined: the av matmul for tile jt is emitted
                # after the scores matmul of jt+3, so the (in-order) PE never
                # waits on the exp -> mask -> padd chain
                pending = []
                LOOKAHEAD = 4

                def _consume(pjt, ppT, pn0, last):
                    # diagonal av matmuls skip the masked-out (zero) query
                    # columns; the untouched psum region keeps its value from
                    # the other jt tiles (skip_group_check: partial-region
                    # start/stop bookkeeping is sim-only)
                    nc.tensor.matmul(ctx_acc[:, pn0:512],
                                     vnat_sb[:, pjt * 128:(pjt + 1) * 128],
                                     ppT[:, pn0:512], start=(pjt == 0), stop=last,
                                     skip_group_check=(pn0 > 0 or last))

                def _emit_padd(pjt, ppT):
                    # row-sum partials: two alternating f32r chains on the
                    # DVE, deferred one jt step so the diagonal mask multiply
                    # (which gates the av matmul) never queues behind them
                    p = padd[pjt % 2]
                    if pjt < 2:
                        nc.vector.tensor_copy(p[:], ppT[:])
                    else:
                        nc.vector.tensor_add(p[:], p[:], ppT[:])

                for jt in range(jt_max + 1):
                    # diagonal tiles only produce scores for queries >= 128*r;
                    # the matmul/exp are trimmed to that range (the mask
                    # multiply still covers the whole tile, zeroing the stale
                    # region). chunk 0 stays full-width so every pT pool
                    # buffer is written before any trimmed use.
                    r = jt - icnk * 4
                    n0 = 128 * r if (icnk > 0 and r >= 1) else 0
                    sT = psum.tile([128, 512], F32, tag="ps")
                    nc.tensor.matmul(sT[:, n0:512], krope_sb[:, jt * 128:(jt + 1) * 128],
                                     qr[:, n0:512], start=True, stop=True)
                    if jt == 1 and _PENDING:
                        _finish_head(*_PENDING.pop())
                    if len(pending) >= LOOKAHEAD:
                        _consume(*pending.pop(0), False)
                    pT = ptp.tile([128, 512], BF16, tag="pt")
                    nc.scalar.activation(out=pT[:, n0:512], in_=sT[:, n0:512],
                                         func=mybir.ActivationFunctionType.Exp,
                                         scale=INV_SQRT_D)
                    if r >= 0:
                        nc.vector.tensor_mul(pT[:], pT[:], mask_sb[:, r, :])
                    if jt > 0:
                        _emit_padd(jt - 1, pending[-1][1])
                    pending.append((jt, pT, n0))
                _emit_padd(jt_max, pT)
                while pending:
                    _consume(*pending.pop(0), len(pending) == 0)
                _PENDING.append((ctx_acc, padd, h, c0, c1))
                if icnk > 0:
                    _emit_oproj_quarter(icnk - 1, h)

        _finish_head(*_PENDING.pop())
        for st in range(4):
            _emit_oproj_quarter(NCHUNK - 1, st)

    nc.finalize()
    return nc


_NC_CACHE = None
_TABLES_CACHE = None


def _host_tables():
    inv_freq = 1.0 / (ROPE_BASE ** (np.arange(0, HD, 2, dtype=np.float32) / HD))
    pos = np.arange(S, dtype=np.float32)
    freqs = pos[:, None] * inv_freq[None, :].astype(np.float32)   # [S, 64]
    emb = np.concatenate([freqs, freqs], axis=1).astype(np.float32)  # [S, 128]
    cosT = np.ascontiguousarray(np.cos(emb).T).astype(BF)  # [128, S]
    sinT = np.ascontiguousarray(np.sin(emb).T).astype(BF)

    # multiplicative causal mask for diagonal blocks, transposed [jp, r, if]
    jp = np.arange(128)[:, None, None]
    r = np.arange(4)[None, :, None]
    iF = np.arange(512)[None, None, :]
    mask01 = (r * 128 + jp <= iF).astype(np.float32).astype(BF)

    rt = np.zeros((128, 128), dtype=np.float32)
    idx = np.arange(64)
    rt[idx + 64, idx] = -1.0
    rt[idx, idx + 64] = 1.0
    rt = rt.astype(BF)

    ident = np.eye(128, dtype=np.float32)
    ones = np.ones((128, 128), dtype=np.float32)
    return cosT, sinT, mask01, rt, ident, ones


def kernel(hidden_states, Wq, Wk, Wv, Wo):
    global _NC_CACHE, _TABLES_CACHE
    if _NC_CACHE is None:
        _NC_CACHE = _build_nc()
    nc = _NC_CACHE
    if _TABLES_CACHE is None:
        _TABLES_CACHE = _host_tables()
    cosT, sinT, mask01, rt, ident, ones = _TABLES_CACHE

    hs = np.asarray(hidden_states, dtype=np.float32)
    B = hs.shape[0]
    assert hs.shape == (B, S, D)
    hst = np.ascontiguousarray(hs[0].T).astype(BF)  # [D, S]

    Wq = np.asarray(Wq, dtype=np.float32)
    Wk = np.asarray(Wk, dtype=np.float32)
    Wv = np.asarray(Wv, dtype=np.float32)
    Wo = np.asarray(Wo, dtype=np.float32)

    in_maps = []
    for c in range(NCORES):
        in_maps.append({
            "hst": hst,
            "wq": np.ascontiguousarray(Wq[:, c * QH * HD:(c + 1) * QH * HD]).astype(BF),
            "wk": np.ascontiguousarray(Wk[:, c * HD:(c + 1) * HD]).astype(BF),
            "wv": np.ascontiguousarray(Wv[:, c * HD:(c + 1) * HD]).astype(BF),
            "wo": np.ascontiguousarray(Wo[c * QH * HD:(c + 1) * QH * HD, :]).astype(BF),
            "cost": cosT,
            "sint": sinT,
            "maskt": mask01,
            "rt": rt,
            "ident": ident,
            "ones": ones,
        })

    import os
    trace = os.environ.get("KERNEL_TRACE") == "1"
    if trace:
        try:
            import antenv.axon_hooks  # noqa: F401  (profiling hook, optional)
        except ImportError:
            trace = False
    res = run_bass_kernel_spmd(nc, in_maps, list(range(NCORES)), trace=trace)
    if trace:
        kernel.last_results = res

    acc = np.zeros((S, D), dtype=np.float32)
    for c in range(NCORES):
        acc += res.results[c]["out"].astype(np.float32)
    return acc.reshape(B, S, D)
